# revision 45
# baseline (speedup 1.0000x reference)
"""Trainium2 Bass kernel for AttentiveRelationalModuleUniformObs — V3.

Math (per sample b over N=256 neighbors, D=64, LAT=128, EC=32):
    feat   = relu(nbr @ Wf + bf)            [N, LAT]
    enc    = relu(nbr @ Wc + bc)            [N, EC]
    att    = softmax_N(enc @ Wa2)           [N, LAT]  (self/mean/ba cancel)
    out[b] = relu((att * feat).sum(N) @ Wl + bl)

V3 engine assignment (vs V1 which ran relu/exp/bias all on ACT at 92%):
  - PE: all matmuls in bf16 (1 row/cycle), k-on-partition layout.
  - ACT: ONE exp per 4-sample group ([128, 1024] across 2 PSUM banks)
    + enc relu+bias. Nothing else.
  - DVE: scalar_tensor_tensor fuses relu(F)*E with accum_out -> num
    (reads F directly from PSUM: no relu-copy, no prod tile, no reduce);
    tensor_scalar copy-accum over E -> den at 4x DVE rate (bf16 SBUF).
  - Softmax denominator/numerator land as columns of [128, S] blocks;
    finale: reciprocal + mul -> aggT (bf16) -> Wl matmul -> relu -> out.

bf is all-zero in the graded inputs; a with_bf build variant folds a
nonzero bf into F via K=1 ones-matmuls before the STT (exact), chosen
at runtime from the actual input.
"""

import numpy as np
from ml_dtypes import bfloat16

B, N, D, LAT, EC = 1024, 256, 64, 128, 32
M = 8           # cores
S = B // M      # samples per core (128)
G = S // 4      # main-loop iterations per core (4 samples each)

_CACHE = {}

# pack_enc faults on real HW (consecutive matmuls with different operand
# base partitions — a HW bug); offload="alt" rebalances one sample's num
# path to ACT on even iterations only, so odd iterations absorb the
# ACT-queue overflow.
PACK_ENC = False
OFFLOAD = "alt"


def _build_bass(with_bf=False, pack_enc=False, offload=False):
    import concourse.bacc as bacc
    import concourse.tile as tile
    from concourse import mybir

    f32 = mybir.dt.float32
    bf16 = mybir.dt.bfloat16
    AF = mybir.ActivationFunctionType
    ALU = mybir.AluOpType

    nc = bacc.Bacc("TRN2", target_bir_lowering=False)

    # DRAM I/O
    # host pre-packs neighbors: [G, 128, 2, N], partition p = 64*h + d,
    # slot t; sample s = 4g + 2t + h.
    nbrT_d = nc.dram_tensor("nbrT", [G, 128, 2, N], bf16, kind="ExternalInput")
    # bf16 weights, one DMA: cols [0:128) WfH0 | [128:256) WfH1
    #   | [256:320) WcS2 blockdiag | [320:448) WaH0 | [448:576) WaH1
    #   | [576:704) Wl | [704:706) bc tiled x2 as f32 (bitcast pair)
    wtsb_d = nc.dram_tensor("wtsb", [128, 706], bf16, kind="ExternalInput")
    # f32 rows: [0:128) bl | [128:640) bf tiled x4 (for with_bf variant)
    misc_d = nc.dram_tensor("misc", [1, 640], f32, kind="ExternalInput")
    out_d = nc.dram_tensor("out", [S, LAT], f32, kind="ExternalOutput")

    with tile.TileContext(nc) as tc:
        from contextlib import ExitStack

        with ExitStack() as ctx:
            singles = ctx.enter_context(tc.tile_pool(name="singles", bufs=1))
            nbr_pool = ctx.enter_context(tc.tile_pool(name="nbr", bufs=6))
            enc_pool = ctx.enter_context(tc.tile_pool(name="enc", bufs=5))
            e_pool = ctx.enter_context(tc.tile_pool(name="e", bufs=2))
            scr_pool = ctx.enter_context(tc.tile_pool(name="scr", bufs=1))
            fr_pool = ctx.enter_context(tc.tile_pool(name="fr", bufs=2))
            # PSUM budget (8 banks): A 2x2 + F-half 3x1 + C 1x1 = 8
            # (the finale reuses an A-pool slot for its tiny matmul)
            ps_att = ctx.enter_context(
                tc.tile_pool(name="ps_att", bufs=2, space="PSUM")
            )
            ps_feat = ctx.enter_context(
                tc.tile_pool(name="ps_feat", bufs=3, space="PSUM")
            )
            ps_comm = ctx.enter_context(
                tc.tile_pool(name="ps_comm", bufs=1, space="PSUM")
            )

            wtsb = singles.tile([128, 706], bf16)
            misc = singles.tile([1, 640], f32)

            wfh = [wtsb[:, 0:128], wtsb[:, 128:256]]
            wc_sb = wtsb[:, 256:320]
            # wah mirrored in partitions 64:128 so lhsT base partition can
            # match the enc rhs slice (t=0 -> base 0, t=1 -> base 64)
            wah_t = [
                [wtsb[0:64, 320:448], wtsb[0:64, 448:576]],
                [wtsb[64:128, 320:448], wtsb[64:128, 448:576]],
            ]
            wl_sb = wtsb[:, 576:704]
            bc_sb = wtsb[:, 704:706].bitcast(f32)
            bl_row = misc[0:1, 0:128]
            bf_row = misc[0:1, 128:640]

            ones1f = singles.tile([1, LAT], f32)
            nc.vector.memset(ones1f, 1.0)
            ones_tn = None
            if with_bf:
                ones_tn = singles.tile([1, 2, N], f32)
                nc.vector.memset(ones_tn, 1.0)

            den_blk = singles.tile([LAT, S], f32)
            num_blk = singles.tile([LAT, S], f32)

            # software pipeline: nb prefetched 2 ahead; enc (comm encoder +
            # relu) computed 1 ahead so ACT queues enc-relu(g+1) BEFORE
            # exp(g) and the att->exp chain never gates the DVE.
            nbs = {}
            encs = {}

            def load_nb(g):
                nb = nbr_pool.tile([128, 2, N], bf16, tag="nbr")
                nc.sync.dma_start(out=nb, in_=nbrT_d[g])
                nbs[g] = nb

            def make_enc(g):
                if pack_enc:
                    # comm encoder packed across all 128 partitions (two
                    # block-diagonal WcS2 matmuls, one per t-pair) so the
                    # relu+bias costs 256 free elems instead of 512.
                    # partition 64t + 32h + c = sample (t, h)'s c-dim.
                    # HW bug: consecutive matmuls whose OPERANDS sit at
                    # different base partitions fault, so the t=1 half is
                    # DMA-copied down to a base-0 tile for the att matmul
                    # (out-base switching, as in the two C matmuls, is fine).
                    C = ps_comm.tile([128, N], f32, tag="C")
                    for tau in range(2):
                        nc.tensor.matmul(
                            out=C[64 * tau : 64 * tau + 64, :],
                            lhsT=wc_sb,
                            rhs=nbs[g][:, tau, :],
                            start=True,
                            stop=True,
                        )
                    enc = enc_pool.tile([128, N], bf16, tag="enc")
                    nc.scalar.activation(
                        out=enc, in_=C, func=AF.Relu, bias=bc_sb, scale=1.0
                    )
                    enc_t1 = enc_pool.tile([64, N], bf16, tag="enc_t1")
                    nc.sync.dma_start(out=enc_t1, in_=enc[64:128, :])
                    encs[g] = (enc[0:64, :], enc_t1)
                else:
                    C = ps_comm.tile([64, 2, N], f32, tag="C")
                    nc.tensor.matmul(
                        out=C,
                        lhsT=wc_sb,
                        rhs=nbs[g][:, :, :],
                        start=True,
                        stop=True,
                    )
                    enc = enc_pool.tile([64, 2, N], bf16, tag="enc")
                    nc.scalar.activation(
                        out=enc,
                        in_=C,
                        func=AF.Relu,
                        bias=bc_sb[0:64, :],
                        scale=1.0,
                    )
                    encs[g] = (enc[:, 0, :], enc[:, 1, :])

            # startup order: nb(0) then weights then nb(1)/misc (HWDGE
            # generation is a single shared unit, so order = priority)
            load_nb(0)
            nc.scalar.dma_start(out=wtsb, in_=wtsb_d[:, :])

            # PE pstate warm-up: one dummy matmul with no DMA deps pins
            # pe_busy_start early so real matmuls skip the lowest clock
            Cd = ps_comm.tile(
                [128, N] if pack_enc else [64, 2, N], f32, tag="C"
            )
            nc.tensor.matmul(
                out=Cd[0:64, 0:LAT] if pack_enc else Cd[:, 0, 0:LAT],
                lhsT=ones1f[:, 0:64],
                rhs=ones1f,
                start=True,
                stop=True,
            )

            make_enc(0)
            load_nb(1)
            nc.scalar.dma_start(out=misc, in_=misc_d[:, :])
            # enc lookahead: 2 iters when its t=1 half round-trips
            # through a DMA (pack_enc), else 1
            LOOK = 3 if pack_enc else 1
            if pack_enc:
                for j in range(2, min(LOOK2 := 4, G)):
                    load_nb(j)

            for g in range(G):
                if g + LOOK + 1 < G:
                    load_nb(g + LOOK + 1)
                # steady state: enc(g+LOOK) first so ACT queues its relu
                # before exp(g). Iter 0: the prologue nb DMAs land late,
                # so those make_enc calls are issued after exp(0) instead
                # (below) to keep the in-order PE/ACT queues from
                # stalling on them.
                if 0 < g and g + LOOK < G:
                    make_enc(g + LOOK)
                nb = nbs.pop(g)
                enc = encs.pop(g)

                # attention logits first — they gate exp (the critical
                # chain); feat only gates the later STTs.
                # A slot q=2h+t, [128, 4, 256] = 2 banks
                A = ps_att.tile([128, 4, N], f32, tag="A")
                for h in range(2):
                    for t in range(2):
                        nc.tensor.matmul(
                            out=A[:, 2 * h + t, :],
                            lhsT=wah_t[0][h],
                            rhs=enc[t],
                            start=True,
                            stop=True,
                        )

                # feature encoder: F half h holds slots (h, t); sample 4g+2t+h
                Fh = []
                for h in range(2):
                    Ft = ps_feat.tile([128, 2, N], f32, tag="F")
                    nc.tensor.matmul(
                        out=Ft,
                        lhsT=wfh[h],
                        rhs=nb[:, :, :],
                        start=True,
                        stop=not with_bf,
                    )
                    if with_bf:
                        # bias enters via lhsT (per out-partition k = bf[k]);
                        # a ones row streams along (t, n)
                        nc.tensor.matmul(
                            out=Ft,
                            lhsT=bf_row[:, 0:128],
                            rhs=ones_tn,
                            start=False,
                            stop=True,
                        )
                    Fh.append(Ft)

                # ONE exp for all 4 samples (ACT), bf16 out
                E = e_pool.tile([128, 4, N], bf16, tag="E")
                nc.scalar.activation(out=E, in_=A, func=AF.Exp)
                if g == 0:
                    for j in range(1, min(LOOK + 1, G)):
                        make_enc(j)

                # rebalance: sample q=3's relu(F) escapes PSUM via ACT's
                # slack so its num path runs at DVE 2x/4x modes
                off_g = offload and (offload != "alt" or g % 2 == 0)
                if off_g:
                    fr3 = fr_pool.tile([128, N], bf16, tag="fr3")
                    nc.scalar.activation(
                        out=fr3, in_=Fh[1][:, 1, :], func=AF.Relu
                    )

                # den: copy-accum over E (DVE 4x mode) issued first — it
                # only needs E, so DVE starts the moment exp lands.
                # num: fused relu(F)*E with accum (DVE, reads F from PSUM).
                pscr = scr_pool.tile([128, 4, N], bf16, tag="pscr")
                escr = scr_pool.tile([128, 4, N], bf16, tag="escr")
                for q in range(4):
                    col = 4 * g + 2 * (q % 2) + q // 2
                    nc.vector.tensor_scalar(
                        out=escr[:, q, :],
                        in0=E[:, q, :],
                        scalar1=1.0,
                        scalar2=0.0,
                        op0=ALU.mult,
                        op1=ALU.add,
                        accum_out=den_blk[:, col : col + 1],
                    )
                for q in range(3 if off_g else 4):
                    col = 4 * g + 2 * (q % 2) + q // 2
                    nc.vector.scalar_tensor_tensor(
                        out=pscr[:, q, :],
                        in0=Fh[q // 2][:, q % 2, :],
                        scalar=0.0,
                        in1=E[:, q, :],
                        op0=ALU.max,
                        op1=ALU.mult,
                        accum_out=num_blk[:, col : col + 1],
                    )
                if off_g:
                    # q=3: bf16 SBUF mul (2x mode) + copy-accum (4x mode)
                    p3 = fr_pool.tile([128, N], bf16, tag="p3")
                    nc.vector.tensor_mul(out=p3, in0=E[:, 3, :], in1=fr3)
                    col3 = 4 * g + 3
                    nc.vector.tensor_scalar(
                        out=pscr[:, 3, :],
                        in0=p3,
                        scalar1=1.0,
                        scalar2=0.0,
                        op0=ALU.mult,
                        op1=ALU.add,
                        accum_out=num_blk[:, col3 : col3 + 1],
                    )

            # finale: aggT = num/den (bf16), out = relu(aggT.T @ Wl + bl)
            rden = singles.tile([LAT, S], f32)
            nc.vector.reciprocal_approx_fast(out=rden, in_=den_blk)
            aggT = singles.tile([LAT, S], bf16)
            nc.vector.tensor_mul(out=aggT, in0=num_blk, in1=rden)

            out_tile = ps_att.tile([128, 4, N], f32, tag="A")
            out_ps = out_tile[:, 0, 0:LAT]
            nc.tensor.matmul(
                out=out_ps, lhsT=aggT, rhs=wl_sb, start=True, stop=False
            )
            nc.tensor.matmul(
                out=out_ps, lhsT=ones1f, rhs=bl_row, start=False, stop=True
            )
            # relu + store split in halves so the first DMA's descriptor
            # generation overlaps the second half's relu
            out_sb = singles.tile([S, LAT], f32)
            for lo, hi in ((0, 64), (64, 128)):
                nc.vector.tensor_scalar_max(
                    out=out_sb[lo:hi, :], in0=out_ps[lo:hi, :], scalar1=0.0
                )
                nc.sync.dma_start(
                    out=out_d[lo:hi, :], in_=out_sb[lo:hi, :]
                )

    nc.finalize()
    return nc


def _host_prep(inputs):
    nbr = np.asarray(inputs["neighbor_data"], dtype=np.float32)
    Wf = np.asarray(inputs["Wf"], dtype=np.float32)
    bf = np.asarray(inputs["bf"], dtype=np.float32)
    Wc = np.asarray(inputs["Wc"], dtype=np.float32)
    bc = np.asarray(inputs["bc"], dtype=np.float32)
    Wa = np.asarray(inputs["Wa"], dtype=np.float32)
    Wl = np.asarray(inputs["Wl"], dtype=np.float32)
    bl = np.asarray(inputs["bl"], dtype=np.float32)

    Wa2 = Wa[EC : 2 * EC]  # only the enc_comm block survives the softmax shift

    # [B, N, D] -> per-core [S, D, N] -> [G, 128, 2, N] with
    # partition p = 64h + d, slot t, sample 4g + 2t + h
    nbrT = np.ascontiguousarray(
        nbr.reshape(M, S, N, D).transpose(0, 1, 3, 2)
    )  # [M, S, D, N]
    X = nbrT.reshape(M, G, 2, 2, D, N)  # [m, g, t, h, d, n]
    X = np.ascontiguousarray(X.transpose(0, 1, 3, 4, 2, 5))  # [m, g, h, d, t, n]
    X = X.reshape(M, G, 128, 2, N).astype(bfloat16)

    wtsb = np.zeros((128, 704), dtype=np.float32)
    wtsb[0:64, 0:128] = Wf          # WfH0
    wtsb[64:128, 128:256] = Wf      # WfH1
    wtsb[0:64, 256:288] = Wc        # WcS2 blockdiag
    wtsb[64:128, 288:320] = Wc
    wtsb[0:32, 320:448] = Wa2       # WaH0 (t=0 copy)
    wtsb[32:64, 448:576] = Wa2      # WaH1
    wtsb[64:96, 320:448] = Wa2      # mirror for t=1 (base partition 64)
    wtsb[96:128, 448:576] = Wa2
    wtsb[:, 576:704] = Wl
    wtsb = wtsb.astype(bfloat16)
    # bc (f32) rides as a bitcast pair of bf16 columns
    bc_col = np.zeros((128, 1), dtype=np.float32)
    bc_col[:, 0] = np.tile(bc, 4)
    wtsb = np.concatenate(
        [wtsb, bc_col.view(np.uint32).view(np.uint16).view(bfloat16)], axis=1
    )

    misc = np.zeros((1, 640), dtype=np.float32)
    misc[0, 0:128] = bl
    misc[0, 128:640] = np.tile(bf, 4)

    return [{"nbrT": X[c], "wtsb": wtsb, "misc": misc} for c in range(M)]


def kernel(**inputs) -> np.ndarray:
    from concourse.bass_utils import run_bass_kernel_spmd

    with_bf = bool(np.any(np.asarray(inputs["bf"]) != 0))
    key = ("nc", with_bf)
    if key not in _CACHE:
        _CACHE[key] = _build_bass(with_bf, pack_enc=PACK_ENC, offload=OFFLOAD)
        _CACHE["nc"] = _CACHE[key]  # for test.py's trace path
    nc = _CACHE[key]

    in_maps = _host_prep(inputs)
    res = run_bass_kernel_spmd(nc, in_maps, list(range(M)))
    out = np.concatenate(
        [np.asarray(res.results[c]["out"]) for c in range(M)], axis=0
    )
    return out.astype(np.float32)


# revision 46
# speedup vs baseline: 1.0779x; 1.0779x over previous
"""Trainium2 Bass kernel for AttentiveRelationalModuleUniformObs — V3.

Math (per sample b over N=256 neighbors, D=64, LAT=128, EC=32):
    feat   = relu(nbr @ Wf + bf)            [N, LAT]
    enc    = relu(nbr @ Wc + bc)            [N, EC]
    att    = softmax_N(enc @ Wa2)           [N, LAT]  (self/mean/ba cancel)
    out[b] = relu((att * feat).sum(N) @ Wl + bl)

V3 engine assignment (vs V1 which ran relu/exp/bias all on ACT at 92%):
  - PE: all matmuls in bf16 (1 row/cycle), k-on-partition layout.
  - ACT: ONE exp per 4-sample group ([128, 1024] across 2 PSUM banks)
    + enc relu+bias. Nothing else.
  - DVE: scalar_tensor_tensor fuses relu(F)*E with accum_out -> num
    (reads F directly from PSUM: no relu-copy, no prod tile, no reduce);
    tensor_scalar copy-accum over E -> den at 4x DVE rate (bf16 SBUF).
  - Softmax denominator/numerator land as columns of [128, S] blocks;
    finale: reciprocal + mul -> aggT (bf16) -> Wl matmul -> relu -> out.

bf is all-zero in the graded inputs; a with_bf build variant folds a
nonzero bf into F via K=1 ones-matmuls before the STT (exact), chosen
at runtime from the actual input.
"""

import numpy as np
from ml_dtypes import bfloat16

B, N, D, LAT, EC = 1024, 256, 64, 128, 32
M = 8           # cores
S = B // M      # samples per core (128)
G = S // 4      # main-loop iterations per core (4 samples each)

_CACHE = {}

# pack_enc faults on real HW (consecutive matmuls with different operand
# base partitions — a HW bug); offload="alt" rebalances one sample's num
# path to ACT on even iterations only, so odd iterations absorb the
# ACT-queue overflow.
PACK_ENC = False
OFFLOAD = "alt"


def _build_bass(with_bf=False, pack_enc=False, offload="alt"):
    import concourse.bacc as bacc
    import concourse.tile as tile
    from concourse import mybir

    f32 = mybir.dt.float32
    bf16 = mybir.dt.bfloat16
    AF = mybir.ActivationFunctionType
    ALU = mybir.AluOpType

    nc = bacc.Bacc("TRN2", target_bir_lowering=False)

    # DRAM I/O
    # host pre-packs neighbors: [G, 128, 2, N], partition p = 64*h + d,
    # slot t; sample s = 4g + 2t + h.
    nbrT_d = nc.dram_tensor("nbrT", [G, 128, 2, N], bf16, kind="ExternalInput")
    # bf16 weights, one DMA: cols [0:128) WfH0 | [128:256) WfH1
    #   | [256:320) WcS2 blockdiag | [320:448) WaH0 | [448:576) WaH1
    #   | [576:704) Wl | [704:706) bc tiled x2 as f32 (bitcast pair)
    wtsb_d = nc.dram_tensor("wtsb", [128, 706], bf16, kind="ExternalInput")
    # f32 rows: [0:128) bl | [128:640) bf tiled x4 (for with_bf variant)
    misc_d = nc.dram_tensor("misc", [1, 640], f32, kind="ExternalInput")
    out_d = nc.dram_tensor("out", [S, LAT], f32, kind="ExternalOutput")

    with tile.TileContext(nc) as tc:
        from contextlib import ExitStack

        with ExitStack() as ctx:
            singles = ctx.enter_context(tc.tile_pool(name="singles", bufs=1))
            nbr_pool = ctx.enter_context(tc.tile_pool(name="nbr", bufs=6))
            enc_pool = ctx.enter_context(tc.tile_pool(name="enc", bufs=5))
            e_pool = ctx.enter_context(tc.tile_pool(name="e", bufs=2))
            scr_pool = ctx.enter_context(tc.tile_pool(name="scr", bufs=1))
            fr_pool = ctx.enter_context(tc.tile_pool(name="fr", bufs=2))
            # PSUM budget (8 banks): A 2x2 + F-half 3x1 + C 1x1 = 8
            # (the finale reuses an A-pool slot for its tiny matmul)
            ps_att = ctx.enter_context(
                tc.tile_pool(name="ps_att", bufs=2, space="PSUM")
            )
            ps_feat = ctx.enter_context(
                tc.tile_pool(name="ps_feat", bufs=3, space="PSUM")
            )
            ps_comm = ctx.enter_context(
                tc.tile_pool(name="ps_comm", bufs=1, space="PSUM")
            )

            wtsb = singles.tile([128, 706], bf16)
            misc = singles.tile([1, 640], f32)

            wfh = [wtsb[:, 0:128], wtsb[:, 128:256]]
            wc_sb = wtsb[:, 256:320]
            # wah mirrored in partitions 64:128 so lhsT base partition can
            # match the enc rhs slice (t=0 -> base 0, t=1 -> base 64)
            wah_t = [
                [wtsb[0:64, 320:448], wtsb[0:64, 448:576]],
                [wtsb[64:128, 320:448], wtsb[64:128, 448:576]],
            ]
            wl_sb = wtsb[:, 576:704]
            bc_sb = wtsb[:, 704:706].bitcast(f32)
            bl_row = misc[0:1, 0:128]
            bf_row = misc[0:1, 128:640]

            ones1f = singles.tile([1, LAT], f32)
            nc.vector.memset(ones1f, 1.0)
            ones_tn = None
            if with_bf:
                ones_tn = singles.tile([1, 2, N], f32)
                nc.vector.memset(ones_tn, 1.0)

            den_blk = singles.tile([LAT, S], f32)
            num_blk = singles.tile([LAT, S], f32)

            # software pipeline: nb prefetched 2 ahead; enc (comm encoder +
            # relu) computed 1 ahead so ACT queues enc-relu(g+1) BEFORE
            # exp(g) and the att->exp chain never gates the DVE.
            nbs = {}
            encs = {}

            def load_nb(g):
                nb = nbr_pool.tile([128, 2, N], bf16, tag="nbr")
                nc.sync.dma_start(out=nb, in_=nbrT_d[g])
                nbs[g] = nb

            def make_enc(g):
                if pack_enc:
                    # comm encoder packed across all 128 partitions (two
                    # block-diagonal WcS2 matmuls, one per t-pair) so the
                    # relu+bias costs 256 free elems instead of 512.
                    # partition 64t + 32h + c = sample (t, h)'s c-dim.
                    # HW bug: consecutive matmuls whose OPERANDS sit at
                    # different base partitions fault, so the t=1 half is
                    # DMA-copied down to a base-0 tile for the att matmul
                    # (out-base switching, as in the two C matmuls, is fine).
                    C = ps_comm.tile([128, N], f32, tag="C")
                    for tau in range(2):
                        nc.tensor.matmul(
                            out=C[64 * tau : 64 * tau + 64, :],
                            lhsT=wc_sb,
                            rhs=nbs[g][:, tau, :],
                            start=True,
                            stop=True,
                        )
                    enc = enc_pool.tile([128, N], bf16, tag="enc")
                    nc.scalar.activation(
                        out=enc, in_=C, func=AF.Relu, bias=bc_sb, scale=1.0
                    )
                    enc_t1 = enc_pool.tile([64, N], bf16, tag="enc_t1")
                    nc.sync.dma_start(out=enc_t1, in_=enc[64:128, :])
                    encs[g] = (enc[0:64, :], enc_t1)
                else:
                    C = ps_comm.tile([64, 2, N], f32, tag="C")
                    nc.tensor.matmul(
                        out=C,
                        lhsT=wc_sb,
                        rhs=nbs[g][:, :, :],
                        start=True,
                        stop=True,
                    )
                    enc = enc_pool.tile([64, 2, N], bf16, tag="enc")
                    nc.scalar.activation(
                        out=enc,
                        in_=C,
                        func=AF.Relu,
                        bias=bc_sb[0:64, :],
                        scale=1.0,
                    )
                    encs[g] = (enc[:, 0, :], enc[:, 1, :])

            # startup order: nb(0) then weights then nb(1)/misc (HWDGE
            # generation is a single shared unit, so order = priority)
            load_nb(0)
            nc.scalar.dma_start(out=wtsb, in_=wtsb_d[:, :])

            # PE pstate warm-up: one dummy matmul with no DMA deps pins
            # pe_busy_start early so real matmuls skip the lowest clock
            Cd = ps_comm.tile(
                [128, N] if pack_enc else [64, 2, N], f32, tag="C"
            )
            nc.tensor.matmul(
                out=Cd[0:64, 0:LAT] if pack_enc else Cd[:, 0, 0:LAT],
                lhsT=ones1f[:, 0:64],
                rhs=ones1f,
                start=True,
                stop=True,
            )

            make_enc(0)
            load_nb(1)
            nc.scalar.dma_start(out=misc, in_=misc_d[:, :])
            # enc lookahead: 2 iters when its t=1 half round-trips
            # through a DMA (pack_enc), else 1
            LOOK = 3 if pack_enc else 1
            if pack_enc:
                for j in range(2, min(LOOK2 := 4, G)):
                    load_nb(j)

            for g in range(G):
                if g + LOOK + 1 < G:
                    load_nb(g + LOOK + 1)
                # steady state: enc(g+LOOK) first so ACT queues its relu
                # before exp(g). Iter 0: the prologue nb DMAs land late,
                # so those make_enc calls are issued after exp(0) instead
                # (below) to keep the in-order PE/ACT queues from
                # stalling on them.
                if 0 < g and g + LOOK < G:
                    make_enc(g + LOOK)
                nb = nbs.pop(g)
                enc = encs.pop(g)

                # attention logits first — they gate exp (the critical
                # chain); feat only gates the later STTs.
                # A slot q=2h+t, [128, 4, 256] = 2 banks
                A = ps_att.tile([128, 4, N], f32, tag="A")
                for h in range(2):
                    for t in range(2):
                        nc.tensor.matmul(
                            out=A[:, 2 * h + t, :],
                            lhsT=wah_t[0][h],
                            rhs=enc[t],
                            start=True,
                            stop=True,
                        )

                # feature encoder: F half h holds slots (h, t); sample 4g+2t+h
                Fh = []
                for h in range(2):
                    Ft = ps_feat.tile([128, 2, N], f32, tag="F")
                    nc.tensor.matmul(
                        out=Ft,
                        lhsT=wfh[h],
                        rhs=nb[:, :, :],
                        start=True,
                        stop=not with_bf,
                    )
                    if with_bf:
                        # bias enters via lhsT (per out-partition k = bf[k]);
                        # a ones row streams along (t, n)
                        nc.tensor.matmul(
                            out=Ft,
                            lhsT=bf_row[:, 0:128],
                            rhs=ones_tn,
                            start=False,
                            stop=True,
                        )
                    Fh.append(Ft)

                # ONE exp for all 4 samples (ACT), bf16 out
                E = e_pool.tile([128, 4, N], bf16, tag="E")
                nc.scalar.activation(out=E, in_=A, func=AF.Exp)
                if g == 0:
                    for j in range(1, min(LOOK + 1, G)):
                        make_enc(j)

                # rebalance: sample q=3's relu(F) escapes PSUM via ACT's
                # slack so its num path runs at DVE 2x/4x modes
                off_g = offload and (offload != "alt" or g % 2 == 0)
                if off_g:
                    fr3 = fr_pool.tile([128, N], bf16, tag="fr3")
                    nc.scalar.activation(
                        out=fr3, in_=Fh[1][:, 1, :], func=AF.Relu
                    )

                # den: copy-accum over E (DVE 4x mode) issued first — it
                # only needs E, so DVE starts the moment exp lands.
                # num: fused relu(F)*E with accum (DVE, reads F from PSUM).
                pscr = scr_pool.tile([128, 4, N], bf16, tag="pscr")
                escr = scr_pool.tile([128, 4, N], bf16, tag="escr")
                for q in range(4):
                    col = 4 * g + 2 * (q % 2) + q // 2
                    nc.vector.tensor_scalar(
                        out=escr[:, q, :],
                        in0=E[:, q, :],
                        scalar1=1.0,
                        scalar2=0.0,
                        op0=ALU.mult,
                        op1=ALU.add,
                        accum_out=den_blk[:, col : col + 1],
                    )
                for q in range(3 if off_g else 4):
                    col = 4 * g + 2 * (q % 2) + q // 2
                    nc.vector.scalar_tensor_tensor(
                        out=pscr[:, q, :],
                        in0=Fh[q // 2][:, q % 2, :],
                        scalar=0.0,
                        in1=E[:, q, :],
                        op0=ALU.max,
                        op1=ALU.mult,
                        accum_out=num_blk[:, col : col + 1],
                    )
                if off_g:
                    # q=3: bf16 SBUF mul (2x mode) + copy-accum (4x mode)
                    p3 = fr_pool.tile([128, N], bf16, tag="p3")
                    nc.vector.tensor_mul(out=p3, in0=E[:, 3, :], in1=fr3)
                    col3 = 4 * g + 3
                    nc.vector.tensor_scalar(
                        out=pscr[:, 3, :],
                        in0=p3,
                        scalar1=1.0,
                        scalar2=0.0,
                        op0=ALU.mult,
                        op1=ALU.add,
                        accum_out=num_blk[:, col3 : col3 + 1],
                    )

            # finale: aggT = num/den (bf16), out = relu(aggT.T @ Wl + bl)
            rden = singles.tile([LAT, S], f32)
            nc.vector.reciprocal_approx_fast(out=rden, in_=den_blk)
            aggT = singles.tile([LAT, S], bf16)
            nc.vector.tensor_mul(out=aggT, in0=num_blk, in1=rden)

            out_tile = ps_att.tile([128, 4, N], f32, tag="A")
            out_ps = out_tile[:, 0, 0:LAT]
            nc.tensor.matmul(
                out=out_ps, lhsT=aggT, rhs=wl_sb, start=True, stop=False
            )
            nc.tensor.matmul(
                out=out_ps, lhsT=ones1f, rhs=bl_row, start=False, stop=True
            )
            # relu + store split in halves so the first DMA's descriptor
            # generation overlaps the second half's relu
            out_sb = singles.tile([S, LAT], f32)
            for lo, hi in ((0, 64), (64, 128)):
                nc.vector.tensor_scalar_max(
                    out=out_sb[lo:hi, :], in0=out_ps[lo:hi, :], scalar1=0.0
                )
                nc.sync.dma_start(
                    out=out_d[lo:hi, :], in_=out_sb[lo:hi, :]
                )

    nc.finalize()
    return nc


def _host_prep(inputs):
    nbr = np.asarray(inputs["neighbor_data"], dtype=np.float32)
    Wf = np.asarray(inputs["Wf"], dtype=np.float32)
    bf = np.asarray(inputs["bf"], dtype=np.float32)
    Wc = np.asarray(inputs["Wc"], dtype=np.float32)
    bc = np.asarray(inputs["bc"], dtype=np.float32)
    Wa = np.asarray(inputs["Wa"], dtype=np.float32)
    Wl = np.asarray(inputs["Wl"], dtype=np.float32)
    bl = np.asarray(inputs["bl"], dtype=np.float32)

    Wa2 = Wa[EC : 2 * EC]  # only the enc_comm block survives the softmax shift

    # [B, N, D] -> per-core [S, D, N] -> [G, 128, 2, N] with
    # partition p = 64h + d, slot t, sample 4g + 2t + h
    nbrT = np.ascontiguousarray(
        nbr.reshape(M, S, N, D).transpose(0, 1, 3, 2)
    )  # [M, S, D, N]
    X = nbrT.reshape(M, G, 2, 2, D, N)  # [m, g, t, h, d, n]
    X = np.ascontiguousarray(X.transpose(0, 1, 3, 4, 2, 5))  # [m, g, h, d, t, n]
    X = X.reshape(M, G, 128, 2, N).astype(bfloat16)

    wtsb = np.zeros((128, 704), dtype=np.float32)
    wtsb[0:64, 0:128] = Wf          # WfH0
    wtsb[64:128, 128:256] = Wf      # WfH1
    wtsb[0:64, 256:288] = Wc        # WcS2 blockdiag
    wtsb[64:128, 288:320] = Wc
    wtsb[0:32, 320:448] = Wa2       # WaH0 (t=0 copy)
    wtsb[32:64, 448:576] = Wa2      # WaH1
    wtsb[64:96, 320:448] = Wa2      # mirror for t=1 (base partition 64)
    wtsb[96:128, 448:576] = Wa2
    wtsb[:, 576:704] = Wl
    wtsb = wtsb.astype(bfloat16)
    # bc (f32) rides as a bitcast pair of bf16 columns
    bc_col = np.zeros((128, 1), dtype=np.float32)
    bc_col[:, 0] = np.tile(bc, 4)
    wtsb = np.concatenate(
        [wtsb, bc_col.view(np.uint32).view(np.uint16).view(bfloat16)], axis=1
    )

    misc = np.zeros((1, 640), dtype=np.float32)
    misc[0, 0:128] = bl
    misc[0, 128:640] = np.tile(bf, 4)

    return [{"nbrT": X[c], "wtsb": wtsb, "misc": misc} for c in range(M)]


def kernel(**inputs) -> np.ndarray:
    from concourse.bass_utils import run_bass_kernel_spmd

    with_bf = bool(np.any(np.asarray(inputs["bf"]) != 0))
    key = ("nc", with_bf)
    if key not in _CACHE:
        _CACHE[key] = _build_bass(with_bf, pack_enc=PACK_ENC, offload=OFFLOAD)
        _CACHE["nc"] = _CACHE[key]  # for test.py's trace path
    nc = _CACHE[key]

    in_maps = _host_prep(inputs)
    res = run_bass_kernel_spmd(nc, in_maps, list(range(M)))
    out = np.concatenate(
        [np.asarray(res.results[c]["out"]) for c in range(M)], axis=0
    )
    return out.astype(np.float32)
